# revision 8
# baseline (speedup 1.0000x reference)
# Depthwise causal conv2d (N=2, C=16, H=W=2048, kernel 6x11) on 8 TRN2 cores.
#
# y[b,c,p,q] = sum_{r,s} w[c,r,s] * xm[b,c, p+r-5, q+s-5], xm = tril-masked x,
# y tril-masked.  Sharding: the 32 (b,c) images are independent; 4 per core.
#
# The kernel runs in TRANSPOSED image space: the host ships x^T (and reads
# y^T back), so the banded-Toeplitz contraction runs over the S=11 column
# taps (band with 11 diagonals) and only R=6 accumulating matmuls per tile
# are needed — 6/11 of the tensor-engine columns of the row-space variant:
#   y^T[q, p] = sum_r band_r[k=q'-q0+5, q-q0] * xT[k, p+r-5],
#   band_r[k, m] = w[r, k-m]  (k-m in [0, 11)).
# In transposed space the causal (tril) region becomes triu: tiles strictly
# below the diagonal are never computed or written (output DRAM is
# pre-zeroed); the diagonal tile of each row gets the causal mask applied to
# the input (gpsimd affine_select in SBUF) and to the output (DVE multiply
# with a 0/1 staircase during PSUM evacuation).
import sys

sys.path.insert(0, "/opt/trn_rl_repo")

import numpy as np

import concourse.bacc as bacc
import concourse.mybir as mybir
import concourse.tile as tile
from concourse.bass_utils import run_bass_kernel_spmd

N, C, H, W = 2, 16, 2048, 2048
R, S, PH, PW = 6, 11, 5, 5
NCORES = 8
IPC = (N * C) // NCORES  # images per core
MT = 118  # output rows (q) per row-tile (128 input rows incl. +-5 halo)
NTS = 512  # max output cols per tile (one PSUM bank of fp32)
BANDW = 128  # allocated band width (cols used: M)
F32 = mybir.dt.float32

# Matmul input dtype. Measured per-core kernel time / rel err vs fp32 ref
# (row-space variant): "bf16" 392 us / 2.3e-3, "f16" 474 us / 3.1e-4.
DTYPE_MODE = "bf16"

_NC_CACHE = {}


def _xdt():
    return {
        "f16": mybir.dt.float16,
        "bf16": mybir.dt.bfloat16,
        "f32r": mybir.dt.float32r,
    }[DTYPE_MODE]


def _np_xdt():
    if DTYPE_MODE == "f16":
        return np.dtype(np.float16)
    if DTYPE_MODE == "bf16":
        import ml_dtypes

        return np.dtype(ml_dtypes.bfloat16)
    return np.dtype(np.float32)


def _row_tiles():
    out = []
    q0 = 0
    while q0 < H:
        out.append((q0, min(MT, H - q0)))
        q0 += MT
    return out


def _col_tiles(q0):
    """Column tiles covering p in [q0, W): anchored at the diagonal."""
    out = []
    p0 = q0
    while p0 < W:
        out.append((p0, min(NTS, W - p0)))
        p0 += NTS
    return out


def _build_program(rep=1):
    """One SPMD program: conv of IPC transposed images [W, H] with per-image
    bands.

    rep > 1 wraps the whole body in a hardware loop executing it `rep`
    times — benchmarking only (amplifies kernel time above the fixed
    dispatch overhead of the execution path)."""
    import contextlib

    xdt = _xdt()
    nc = bacc.Bacc("TRN2", target_bir_lowering=False, debug=False,
                   num_devices=NCORES)
    x = nc.dram_tensor("x", [IPC, W, H], xdt, kind="ExternalInput")
    bands = nc.dram_tensor("bands", [IPC, 128, R * BANDW], xdt,
                           kind="ExternalInput")
    y = nc.dram_tensor("y", [IPC, W, H], xdt, kind="ExternalOutput")

    row_tiles = _row_tiles()

    with tile.TileContext(nc) as tc:
        with (
            tc.tile_pool(name="const", bufs=1) as cpool,
            tc.tile_pool(name="xin", bufs=6) as xpool,
            tc.tile_pool(name="out", bufs=4) as opool,
            tc.tile_pool(name="psum", bufs=8, space="PSUM") as ppool,
            tc.For_i(0, rep, 1) if rep > 1 else contextlib.nullcontext(),
        ):
            # Per-image Toeplitz bands, resident for the whole kernel.
            bt = cpool.tile([128, IPC * R * BANDW], xdt)
            for i in range(IPC):
                nc.sync.dma_start(
                    out=bt[:, i * R * BANDW:(i + 1) * R * BANDW],
                    in_=bands[i],
                )
            # Staircase causal mask (triu): stair[i, u] = 1 iff u >= i.
            stair = cpool.tile([128, NTS], F32)
            nc.gpsimd.memset(stair[:], 1.0)
            nc.gpsimd.affine_select(
                out=stair[:], in_=stair[:],
                compare_op=mybir.AluOpType.is_ge, fill=0.0,
                base=0, channel_multiplier=-1,
                pattern=[[1, NTS]],
            )

            for i in range(IPC):
                band_i = bt[:, i * R * BANDW:(i + 1) * R * BANDW]
                for (q0, M) in row_tiles:
                    _emit_strip(nc, tc, xpool, opool, ppool, x, y, band_i,
                                stair, i, q0, M)
    nc.compile()
    return nc


def _emit_strip(nc, tc, xpool, opool, ppool, x, y, band_i, stair, i, q0, M):
    """One row-tile: a single input-strip DMA, all col-tile matmuls off
    SBUF slices, one output-strip DMA. Minimizes HWDGE occupancy (~625ns
    per DMA instruction, a shared serialized resource)."""
    xdt = _xdt()
    # Input strip: rows [q0-5, q0+123), cols [q0-5, 2048), clipped.
    a0 = q0 - PW
    av0, av1 = max(0, a0), min(H, q0 + 123)
    sw = W - a0  # strip width (cols a0..W)
    ow = W - q0  # output strip width (cols q0..W)

    xs = xpool.tile([128, W + PW], xdt, tag="xin")
    if av0 > a0 or av1 - a0 < 128:
        # first/last row-tile: halo rows/cols outside the image would be
        # stale garbage — zero the whole strip before the partial load
        # (gpsimd partition base must be aligned, so no partial memset).
        nc.gpsimd.memset(xs[:, 0:sw], 0.0)
    nc.sync.dma_start(
        out=xs[av0 - a0:av1 - a0, av0 - a0:sw],
        in_=x[i, av0:av1, av0:W],
    )
    # causal (triu in transposed space): keep iff col >= row, i.e.
    # (a0+j) >= (a0+k) <=> j >= k.  Only j < 128 can violate it.
    sel_w = min(128, sw)
    nc.gpsimd.affine_select(
        out=xs[:, 0:sel_w], in_=xs[:, 0:sel_w],
        compare_op=mybir.AluOpType.is_ge, fill=0.0,
        base=0, channel_multiplier=-1,
        pattern=[[1, sel_w]],
    )

    ys = opool.tile([128, W], xdt, tag="out")
    for (p0, nd) in _col_tiles(q0):
        # lhsT is the full 128-wide band: output rows M..127 are partial-tap
        # garbage that is never evacuated, but NumWeights==128 enables the
        # compiler's Fast Weight Load path (2x faster LDWEIGHTS).
        pt = ppool.tile([128, NTS], F32, tag="psum")
        off = p0 - q0  # strip-col offset of this tile's first output col
        for r in range(R):
            nc.tensor.matmul(
                pt[:, :nd],
                lhsT=band_i[:, r * BANDW:(r + 1) * BANDW],
                rhs=xs[:, off + r:off + r + nd],
                start=(r == 0), stop=(r == R - 1),
            )
        if p0 == q0:
            # Diagonal tile: evacuate PSUM through the causal staircase:
            # keep iff (q0+m) <= (p0+n) <=> stair[m, n].
            nc.vector.tensor_mul(
                ys[:M, off:off + nd], pt[:M, :nd], stair[:M, :nd],
            )
        else:
            nc.any.tensor_copy(ys[:M, off:off + nd], pt[:M, :nd])
    nc.sync.dma_start(
        out=y[i, q0:q0 + M, q0:W],
        in_=ys[:M, 0:ow],
    )


def _build_bands(weight):
    """Host-side: per-image banded Toeplitz weights (transposed space).
    bands[img, k, r*BANDW + m] = w[c(img), r, k-m] for k-m in [0, S)."""
    weight = np.asarray(weight, np.float32)
    nimg = N * C
    bands = np.zeros((nimg, 128, R * BANDW), np.float32)
    m = np.arange(BANDW)
    for r in range(R):
        for dlt in range(S):
            valid = m + dlt < 128
            mv = m[valid]
            # band[m+dlt, r*BANDW+m] = w[c, r, dlt]; img = b*C + c.
            bands[:, mv + dlt, r * BANDW + mv] = np.tile(
                weight[:, r, dlt], N)[:, None]
    return bands.astype(_np_xdt())


def _prep_x(x):
    """Host-side: [N,C,H,W] fp32 -> transposed images [N*C, W, H] in the
    matmul dtype."""
    x = np.asarray(x, dtype=np.float32)
    xT = np.ascontiguousarray(
        x.reshape(N * C, H, W).transpose(0, 2, 1)).astype(
        _np_xdt(), copy=False)
    return xT


def kernel(x, weight):
    x = np.asarray(x, dtype=np.float32)
    weight = np.asarray(weight, dtype=np.float32)
    assert x.shape == (N, C, H, W) and weight.shape == (C, R, S)

    if "nc" not in _NC_CACHE:
        _NC_CACHE["nc"] = _build_program()
    nc = _NC_CACHE["nc"]

    xT_imgs = _prep_x(x)
    bands = _build_bands(weight)
    in_maps = [
        {
            "x": xT_imgs[k * IPC:(k + 1) * IPC],
            "bands": bands[k * IPC:(k + 1) * IPC],
        }
        for k in range(NCORES)
    ]
    res = run_bass_kernel_spmd(nc, in_maps, list(range(NCORES)))
    yT = np.concatenate([res.results[k]["y"] for k in range(NCORES)], axis=0)
    # y^T [N*C, W(q), H(p)] -> y [N, C, H(p), W(q)], back to fp32.
    out = np.ascontiguousarray(
        yT.astype(np.float32).transpose(0, 2, 1))
    return out.reshape(N, C, H, W)
